# revision 2
# baseline (speedup 1.0000x reference)
"""BSplineSynapse Trainium2 kernel (8-core tensor-parallel over out_features).

Math: reference computes, with t = clip(|x|, 0, 1), s = 1 - t:
    w(t) = cp0*s^3 + 3*cp1*s^2*t + 3*cp2*s*t^2 + cp3*t^3   (per (o, i))
    out[b, o] = sum_i w[o, i](t[b, i]) * x[b, i]

Rewriting the cubic in the monomial basis of t:
    w(t) = d0 + d1*t + d2*t^2 + d3*t^3
    d0 = cp0, d1 = 3(cp1 - cp0), d2 = 3(cp0 - 2cp1 + cp2),
    d3 = cp3 - cp0 + 3cp1 - 3cp2
so   out = x @ d0^T + (t x) @ d1^T + (t^2 x) @ d2^T + (t^3 x) @ d3^T.

We fold the factor-3 scaling into the moving-side tensors (g1 = 3 t x,
g2 = 3 t^2 x, g3 = 3 t^3 x) so the weight-side transform needs only
subtractions plus one fused scalar_tensor_tensor:
    out = x @ w0^T + g1 @ A^T + g2 @ (B - A)^T + g3 @ (w3/3 - (w0/3 + B))^T
    A = w1 - w0, B = w2 - w1.

Sharding: out_features split 8 ways (128 per core); x replicated. Each core:
  - x^T staged as (1024, 512) in DRAM, loaded to SBUF as 8 chunks of
    (128 i x 512 b) side by side -> (128, 4096)
  - cp_k^T slices staged as (1024, 128) -> SBUF (128, 1024)
  - 32 accumulating matmuls (4 bases x 8 i-chunks) in float32r (full-rate
    single-pass fp32-reduced) into one PSUM bank -> out^T slice (128, 512).
Host concatenates the 8 out^T slices and transposes back.

When the input satisfies 0 <= x <= 1 (true for this problem's uniform[0,1)
inputs), t == x exactly, so g1 = 3x^2 and g3 = 3x^4 are computed on the
Scalar engine as Square(sqrt(3)*x) / Square(g1/sqrt(3)) and only g2 = x*g1
needs the Vector engine. A general fallback path computes t = clip(|x|,0,1)
explicitly. Path choice only inspects the input range; both paths implement
the full reference function on device.
"""

import sys

if "/opt/trn_rl_repo" not in sys.path:
    sys.path.insert(0, "/opt/trn_rl_repo")

import numpy as np

import concourse.bacc as bacc
import concourse.mybir as mybir
from concourse.mybir import ActivationFunctionType as AF
from concourse.mybir import AluOpType as alu
from concourse.tile import TileContext
from concourse.bass_utils import run_bass_kernel_spmd

B = 512          # batch
I = 1024         # in_features
O = 1024         # out_features
NCORES = 8
OS = O // NCORES  # out_features per core = 128
CH = I // 128     # i-chunks of 128 = 8

F32 = mybir.dt.float32
F32R = mybir.dt.float32r
SQ3 = 3.0 ** 0.5

_programs = {}


def _build(fast: bool):
    nc = bacc.Bacc("TRN2", target_bir_lowering=False, debug=False)
    xT = nc.dram_tensor("xT", [I, B], F32, kind="ExternalInput")
    wd = [
        nc.dram_tensor(f"w{k}", [I, OS], F32, kind="ExternalInput")
        for k in range(4)
    ]
    outT = nc.dram_tensor("outT", [OS, B], F32, kind="ExternalOutput")

    with TileContext(nc) as tc:
        with (
            tc.tile_pool(name="p", bufs=1) as pool,
            tc.tile_pool(name="ps", bufs=1, space="PSUM") as pp,
        ):
            # x^T -> SBUF [p, c*B + b]
            xs = pool.tile([128, CH * B], F32R, tag="xs")
            nc.sync.dma_start(
                out=xs[:].rearrange("p (c b) -> p c b", c=CH),
                in_=xT.ap().rearrange("(c p) b -> p c b", p=128).bitcast(F32R),
            )
            # cp_k^T slices -> SBUF [p, c*OS + o]
            w_sb = []
            for k in range(4):
                t = pool.tile([128, CH * OS], F32R if k == 0 else F32, tag=f"w{k}")
                src_ap = wd[k].ap().rearrange("(c p) o -> p c o", p=128)
                if k == 0:
                    src_ap = src_ap.bitcast(F32R)
                nc.sync.dma_start(
                    out=t[:].rearrange("p (c o) -> p c o", c=CH),
                    in_=src_ap,
                )
                w_sb.append(t)

            g1 = pool.tile([128, CH * B], F32R, tag="g1")
            g2 = pool.tile([128, CH * B], F32R, tag="g2")
            g3 = pool.tile([128, CH * B], F32R, tag="g3")
            if fast:
                # t == x: g1 = 3x^2 = Square(sqrt3*x); g3 = 3x^4 = Square(g1/sqrt3)
                nc.scalar.activation(g1[:], xs[:], AF.Square, scale=SQ3)
                nc.scalar.activation(g3[:], g1[:], AF.Square, scale=1.0 / SQ3)
                nc.vector.tensor_mul(g2[:], xs[:], g1[:])  # 3x^3
            else:
                tt = pool.tile([128, CH * B], F32, tag="tt")
                t2 = pool.tile([128, CH * B], F32, tag="t2")
                # t = min(max(|x|, |0|), 1) = clip(|x|, 0, 1)
                nc.vector.tensor_scalar(
                    tt[:], xs[:], 0.0, 1.0, alu.abs_max, alu.min
                )
                nc.scalar.activation(t2[:], tt[:], AF.Square)
                # g1 = (t*3)*x, g2 = (t2*3)*x, g3 = t2*g1
                nc.vector.scalar_tensor_tensor(
                    g1[:], tt[:], 3.0, xs[:], alu.mult, alu.mult
                )
                nc.vector.scalar_tensor_tensor(
                    g2[:], t2[:], 3.0, xs[:], alu.mult, alu.mult
                )
                nc.vector.tensor_mul(g3[:], t2[:], g1[:])

            # weight transform
            D1 = pool.tile([128, CH * OS], F32R, tag="D1")
            Bt = pool.tile([128, CH * OS], F32, tag="Bt")
            D2 = pool.tile([128, CH * OS], F32R, tag="D2")
            E = pool.tile([128, CH * OS], F32, tag="E")
            D3 = pool.tile([128, CH * OS], F32R, tag="D3")
            nc.vector.tensor_sub(D1[:], w_sb[1][:], w_sb[0][:])      # A
            nc.vector.tensor_sub(Bt[:], w_sb[2][:], w_sb[1][:])      # B
            nc.vector.tensor_sub(D2[:], Bt[:], D1[:])                # B - A
            nc.vector.scalar_tensor_tensor(                          # E = w0/3 + B
                E[:], w_sb[0][:], 1.0 / 3.0, Bt[:], alu.mult, alu.add
            )
            nc.vector.scalar_tensor_tensor(                          # D3 = w3/3 - E
                D3[:], w_sb[3][:], 1.0 / 3.0, E[:], alu.mult, alu.subtract
            )

            G = [xs, g1, g2, g3]
            D = [w_sb[0], D1, D2, D3]
            psum = pp.tile([128, B], F32)
            n = 0
            for k in range(4):
                for c in range(CH):
                    nc.tensor.matmul(
                        psum[:],
                        lhsT=D[k][:, c * OS:(c + 1) * OS],
                        rhs=G[k][:, c * B:(c + 1) * B],
                        start=(n == 0),
                        stop=(n == 31),
                    )
                    n += 1

            osb = pool.tile([128, B], F32, tag="osb")
            nc.scalar.copy(osb[:], psum[:])
            nc.sync.dma_start(out=outT.ap(), in_=osb[:])

    nc.compile()
    return nc


def _get_program(fast: bool):
    if fast not in _programs:
        _programs[fast] = _build(fast)
    return _programs[fast]


def kernel(**inputs) -> np.ndarray:
    x = np.ascontiguousarray(np.asarray(inputs["x"], dtype=np.float32))
    cps = [
        np.ascontiguousarray(np.asarray(inputs[f"cp{k}"], dtype=np.float32))
        for k in range(4)
    ]
    fast = bool(x.min() >= 0.0) and bool(x.max() <= 1.0)
    nc = _get_program(fast)

    xT = np.ascontiguousarray(x.T)
    in_maps = []
    for c in range(NCORES):
        sl = slice(c * OS, (c + 1) * OS)
        m = {"xT": xT}
        for k in range(4):
            m[f"w{k}"] = np.ascontiguousarray(cps[k][sl].T)
        in_maps.append(m)

    res = run_bass_kernel_spmd(nc, in_maps, core_ids=list(range(NCORES)))
    outT = np.concatenate(
        [res.results[c]["outT"] for c in range(NCORES)], axis=0
    )
    return np.ascontiguousarray(outT.T)


# revision 4
# speedup vs baseline: 1.1036x; 1.1036x over previous
"""BSplineSynapse Trainium2 kernel (8-core tensor-parallel over out_features).

Math: reference computes, with t = clip(|x|, 0, 1), s = 1 - t:
    w(t) = cp0*s^3 + 3*cp1*s^2*t + 3*cp2*s*t^2 + cp3*t^3   (per (o, i))
    out[b, o] = sum_i w[o, i](t[b, i]) * x[b, i]

Rewriting the cubic in the monomial basis of t:
    w(t) = d0 + d1*t + d2*t^2 + d3*t^3
    d0 = cp0, d1 = 3(cp1 - cp0), d2 = 3(cp0 - 2cp1 + cp2),
    d3 = cp3 - cp0 + 3cp1 - 3cp2
so   out = x @ d0^T + (t x) @ d1^T + (t^2 x) @ d2^T + (t^3 x) @ d3^T.

We fold the factor-3 scaling into the moving-side tensors (g1 = 3 t x,
g2 = 3 t^2 x, g3 = 3 t^3 x), so on the weight side:
    out = x @ w0^T + g1 @ A^T + g2 @ D2^T + g3 @ D3^T
    A = w1 - w0, B = w2 - w1, D2 = B - A, D3 = w3/3 - (w0/3 + B).

Sharding: out_features split 8 ways (128 per core); x replicated. Each core:
  - x^T pre-permuted on host to SBUF layout (128, 8*512): [p, c*512+b] =
    x[b, c*128+p]; loaded as two halves so compute starts at half-arrival.
  - cp_k^T slices pre-permuted to (128, 8*128): [p, c*128+o] =
    cp_k[o + 128*core, c*128+p].
  - 32 accumulating matmuls (4 bases x 8 i-chunks of K=128, N=512) in
    float32r (single-pass reduced-precision fp32, full PE rate) into one
    PSUM bank -> out^T slice (128, 512).
Host concatenates the 8 out^T slices and transposes back.

When the input satisfies 0 <= x <= 1 (true for this problem's uniform[0,1)
inputs), t == x exactly, so g1 = 3x^2 = Square(sqrt3*x) and
g3 = 3x^4 = Square(g1/sqrt3) run on the Scalar engine and only
g2 = x*g1 needs the Vector engine. A general fallback path computes
t = clip(|x|,0,1) explicitly. Path choice only inspects the input range;
both paths implement the full reference function on device.
"""

import sys

if "/opt/trn_rl_repo" not in sys.path:
    sys.path.insert(0, "/opt/trn_rl_repo")

import numpy as np

import concourse.bacc as bacc
import concourse.mybir as mybir
from concourse.mybir import ActivationFunctionType as AF
from concourse.mybir import AluOpType as alu
from concourse.tile import TileContext
from concourse.bass_utils import run_bass_kernel_spmd

B = 512          # batch
I = 1024         # in_features
O = 1024         # out_features
NCORES = 8
OS = O // NCORES  # out_features per core = 128
CH = I // 128     # i-chunks of 128 = 8
HB = (CH // 2) * B  # free-dim columns per x half = 2048

F32 = mybir.dt.float32
F32R = mybir.dt.float32r
SQ3 = 3.0 ** 0.5

_programs = {}


def _build(fast: bool):
    nc = bacc.Bacc("TRN2", target_bir_lowering=False, debug=False)
    xd = [
        nc.dram_tensor(f"x{h}", [128, HB], F32, kind="ExternalInput")
        for h in range(2)
    ]
    wd = [
        nc.dram_tensor(f"w{k}", [128, CH * OS], F32, kind="ExternalInput")
        for k in range(4)
    ]
    outT = nc.dram_tensor("outT", [OS, B], F32, kind="ExternalOutput")

    with TileContext(nc) as tc:
        with (
            tc.tile_pool(name="p", bufs=1) as pool,
            tc.tile_pool(name="ps", bufs=1, space="PSUM") as pp,
        ):
            # input DMAs, in arrival-priority order
            xs = []
            for h in range(2):
                t = pool.tile([128, HB], F32R, tag=f"x{h}", name=f"x{h}")
                nc.sync.dma_start(out=t[:], in_=xd[h].ap().bitcast(F32R))
                xs.append(t)
            w_sb = []
            for k in range(4):
                dt = F32R if k == 0 else F32
                t = pool.tile([128, CH * OS], dt, tag=f"w{k}", name=f"w{k}")
                src = wd[k].ap()
                if k == 0:
                    src = src.bitcast(F32R)
                nc.sync.dma_start(out=t[:], in_=src)
                w_sb.append(t)

            # x-side basis tensors, per half
            g1 = [pool.tile([128, HB], F32R, tag=f"g1{h}", name=f"g1{h}") for h in range(2)]
            g2 = [pool.tile([128, HB], F32R, tag=f"g2{h}", name=f"g2{h}") for h in range(2)]
            g3 = [pool.tile([128, HB], F32R, tag=f"g3{h}", name=f"g3{h}") for h in range(2)]
            if fast:
                for h in range(2):
                    # g1 = 3x^2, g3 = 3x^4 = (g1)^2/3, g2 = x*g1 = 3x^3
                    nc.scalar.activation(g1[h][:], xs[h][:], AF.Square, scale=SQ3)
                    nc.scalar.activation(
                        g3[h][:], g1[h][:], AF.Square, scale=1.0 / SQ3
                    )
                    nc.vector.tensor_mul(g2[h][:], xs[h][:], g1[h][:])
            else:
                for h in range(2):
                    tt = pool.tile([128, HB], F32, tag=f"tt{h}")
                    t2 = pool.tile([128, HB], F32, tag=f"t2{h}")
                    # t = clip(|x|, 0, 1)
                    nc.vector.tensor_scalar(
                        tt[:], xs[h][:], 0.0, 1.0, alu.abs_max, alu.min
                    )
                    nc.scalar.activation(t2[:], tt[:], AF.Square)
                    nc.vector.scalar_tensor_tensor(
                        g1[h][:], tt[:], 3.0, xs[h][:], alu.mult, alu.mult
                    )
                    nc.vector.scalar_tensor_tensor(
                        g2[h][:], t2[:], 3.0, xs[h][:], alu.mult, alu.mult
                    )
                    nc.vector.tensor_mul(g3[h][:], t2[:], g1[h][:])

            # weight transform
            At = pool.tile([128, CH * OS], F32R, tag="At")
            Bt = pool.tile([128, CH * OS], F32, tag="Bt")
            D2 = pool.tile([128, CH * OS], F32R, tag="D2")
            Et = pool.tile([128, CH * OS], F32, tag="Et")
            D3 = pool.tile([128, CH * OS], F32R, tag="D3")
            # A and D2 on GpSimd (free engine; DVE is the serial bottleneck)
            nc.gpsimd.tensor_sub(At[:], w_sb[1][:], w_sb[0][:])       # A
            nc.vector.tensor_sub(Bt[:], w_sb[2][:], w_sb[1][:])       # B
            nc.gpsimd.tensor_sub(D2[:], Bt[:], At[:])                 # B - A
            nc.vector.scalar_tensor_tensor(                           # E = w0/3 + B
                Et[:], w_sb[0][:], 1.0 / 3.0, Bt[:], alu.mult, alu.add
            )
            nc.vector.scalar_tensor_tensor(                           # D3 = w3/3 - E
                D3[:], w_sb[3][:], 1.0 / 3.0, Et[:], alu.mult, alu.subtract
            )

            G = [xs, g1, g2, g3]
            D = [w_sb[0], At, D2, D3]
            psum = pp.tile([128, B], F32)
            n = 0
            for k in range(4):
                for h in range(2):
                    for c in range(CH // 2):
                        nc.tensor.matmul(
                            psum[:],
                            lhsT=D[k][:, (h * 4 + c) * OS:(h * 4 + c + 1) * OS],
                            rhs=G[k][h][:, c * B:(c + 1) * B],
                            start=(n == 0),
                            stop=(n == 31),
                        )
                        n += 1

            osb = pool.tile([128, B], F32, tag="osb")
            nc.scalar.copy(osb[:], psum[:])
            nc.sync.dma_start(out=outT.ap(), in_=osb[:])

    nc.compile()
    return nc


def _get_program(fast: bool):
    if fast not in _programs:
        _programs[fast] = _build(fast)
    return _programs[fast]


def _stage_x(x):
    # [p, c*512+b] = x[b, c*128+p]; split into halves (chunks 0-3 / 4-7)
    xt = x.T.reshape(CH, 128, B).transpose(1, 0, 2).reshape(128, CH * B)
    return (
        np.ascontiguousarray(xt[:, :HB]),
        np.ascontiguousarray(xt[:, HB:]),
    )


def _stage_w(cp, core):
    # [p, c*128+o] = cp[o + OS*core, c*128+p]
    sl = cp[core * OS:(core + 1) * OS].T  # (1024, 128) [i, o]
    return np.ascontiguousarray(
        sl.reshape(CH, 128, OS).transpose(1, 0, 2).reshape(128, CH * OS)
    )


def make_in_maps(inputs):
    x = np.ascontiguousarray(np.asarray(inputs["x"], dtype=np.float32))
    cps = [
        np.ascontiguousarray(np.asarray(inputs[f"cp{k}"], dtype=np.float32))
        for k in range(4)
    ]
    xA, xB = _stage_x(x)
    in_maps = []
    for c in range(NCORES):
        m = {"x0": xA, "x1": xB}
        for k in range(4):
            m[f"w{k}"] = _stage_w(cps[k], c)
        in_maps.append(m)
    return in_maps


def kernel(**inputs) -> np.ndarray:
    x = np.asarray(inputs["x"], dtype=np.float32)
    fast = bool(x.min() >= 0.0) and bool(x.max() <= 1.0)
    nc = _get_program(fast)
    in_maps = make_in_maps(inputs)
    res = run_bass_kernel_spmd(nc, in_maps, core_ids=list(range(NCORES)))
    outT = np.concatenate(
        [res.results[c]["outT"] for c in range(NCORES)], axis=0
    )
    return np.ascontiguousarray(outT.T)
